# revision 48
# baseline (speedup 1.0000x reference)
"""EqualizedModulatedConv2d (StyleGAN2-style modulated conv) on 8 Trainium2 cores.

Reference computation (per sample n):
    mod[n, ic]  = (style[n] @ fc_weight.T) * FC_SCALER + fc_bias + 1
    w[n]        = WEIGHT_SCALER * weight * mod[n, :, None, None]          # [oC, iC, 3, 3]
    demod[n,oc] = rsqrt(sum_{ic,kh,kw} w^2 + 1e-8)
    out[n]      = conv2d(x[n], w[n] * demod[n, :, None, None, None], pad=1)

Identities used on device: the conv is linear, so
    out[n, oc] = (WEIGHT_SCALER * demod[n, oc]) * conv2d(x[n] * mod[n, ic], weight)
with
    WEIGHT_SCALER * demod[n, oc] = 1 / sqrt(sumsq[n, oc] + 1e-8 / WEIGHT_SCALER^2)
    sumsq[n, oc] = sum_ic A[ic, oc] * mod[n, ic]^2,   A[ic, oc] = sum_taps weight^2

The conv itself runs as 1-D Winograd F(2,3) along H: for output row-pair ty
(rows 2ty, 2ty+1) and padded-image rows d_i = xpad[2ty+i] (i = 0..3),
    V0 = d0 - d2, V1 = d1 + d2, V2 = d2 - d1, V3 = d1 - d3          (DVE)
    M(u)[oc, ty, x] = sum_{ic, kx} U(u,kx)[ic, oc] * V(u)[ic, ty, x+kx]   (PE)
    out[2ty+0] = M0 + M1 + M2,  out[2ty+1] = M1 - M2 - M3           (ACT+DVE)
where U(u,kx) = sum_ky G[u,ky] w[oc, ic, ky, kx] (G = F(2,3) weight
transform) is pre-packed on the host.  12 shifted matmuls per 2 output
rows instead of 18 -> 1.5x less PE work than direct conv (the PE streams
1 elem/cycle regardless of dtype, so MAC count is the only lever).
Everything flows in fp16; PSUM accumulates fp32.  The output transform
consumes PSUM incrementally (ACT mul for the first use of each M plane,
DVE scalar_tensor_tensor for the accumulating uses) so no M staging is
needed and the drain tail is short.

Sharding: data-parallel over N (16 samples / 8 cores = 2 per core);
weights replicated per core.  DMAs are batched (>=0.5 MB) and split
across the two HWDGE issue queues (sync + scalar) to dodge the ~0.64us
per-issue serialization observed in the trace.
"""

import numpy as np

import concourse.bass as bass
import concourse.tile as tile
from concourse import bacc, mybir
import concourse.bass_utils as bass_utils

# keep profiling artifacts local — no S3 in the sandbox
bass_utils.upload_artifacts = lambda tmpdir: "local://" + str(tmpdir)

# ---- problem constants (hardcoded per the harness contract) ----
N, IC, OC, K, SDIM, H, W = 16, 512, 512, 3, 512, 32, 32
N_CORES = 8
NPC = N // N_CORES            # samples per core = 2
PC = IC // 128                # ic chunks = 4
OCC = OC // 128               # oc chunks = 4
SC = SDIM // 128              # sdim chunks = 4
NTAP = 12                     # 4 winograd u-taps x 3 kx taps
NU = 4                        # winograd taps along H
TY = H // 2                   # output row-pair tiles = 16
HP, WP = H + 2, W + 2         # 34, 34 padded
FC_SCALER = 1.0 / np.sqrt(SDIM)
WEIGHT_SCALER = 1.0 / np.sqrt(IC * K * K)
DEMOD_EPS = 1e-8 / (WEIGHT_SCALER * WEIGHT_SCALER)   # 1e-8 * IC * K * K

MODE = "wino1d-fp16-v18"

_NC_CACHE = {}
LAST_RESULT = None  # test.py reads exec_time_ns off this


def build_nc():
    if "nc" in _NC_CACHE:
        return _NC_CACHE["nc"]

    f32 = mybir.dt.float32
    f16 = mybir.dt.float16

    nc = bacc.Bacc("TRN2", target_bir_lowering=False, debug=False,
                   num_devices=N_CORES)

    x = nc.dram_tensor("x", [NPC, IC, H, W], f16, kind="ExternalInput").ap()
    # pk: per-partition [fcw(sc,i) 2048 | st(sc,n) 8 | fcb-as-f32-halves 8]
    pk = nc.dram_tensor("pk", [128, 2064], f16, kind="ExternalInput").ap()
    ut = nc.dram_tensor("ut", [OCC, NU, IC, K, 128], f16, kind="ExternalInput").ap()
    at = nc.dram_tensor("at", [IC, OC], f16, kind="ExternalInput").ap()
    y = nc.dram_tensor("y", [OCC, 128, NPC, TY, 2, W], f16,
                       kind="ExternalOutput").ap()

    xr = x.rearrange("n (c p) g w -> c p n g w", p=128)
    ur = ut.rearrange("o u (c p) t q -> o p u c t q", p=128)
    ar = at.rearrange("(c p) o -> p c o", p=128)

    with tile.TileContext(nc) as tc:
        import contextlib
        with contextlib.ExitStack() as ctx:
            singles = ctx.enter_context(tc.tile_pool(name="singles", bufs=1))
            ypool = ctx.enter_context(tc.tile_pool(name="ypool", bufs=2))
            psc = ctx.enter_context(tc.tile_pool(name="psc", bufs=3, space="PSUM"))
            pss = ctx.enter_context(tc.tile_pool(name="pss", bufs=2, space="PSUM"))

            # ---- persistent SBUF tensors ----
            pk_sb = singles.tile([128, 2064], f16)
            modT_sb = singles.tile([128, PC, NPC], f32)
            mod2T_sb = singles.tile([128, PC, NPC], f16)
            A_sb = singles.tile([128, PC, OC], f16)
            demodT_sb = singles.tile([128, OCC, NPC], f32)
            demodNT_sb = singles.tile([128, OCC, NPC], f32)
            u_sb = singles.tile([128, OCC, NU, PC, K, 128], f16)
            fbv = pk_sb[:, 2056:2064].bitcast(f32)
            # padded modulated image, flat rows (views refactor as needed)
            xpad = singles.tile([128, PC, NPC, HP, WP], f16)
            # winograd-transformed input: [c][u][n][ty][col]
            v_sb = singles.tile([128, PC, NU, NPC, TY, WP], f16)
            eps_sb = singles.tile([128, 1], f32)
            sqd_sb = singles.tile([128, 1], f32)
            sq_sb = singles.tile([128, OCC, NPC], f32)
            warm_sb = singles.tile([128, 512], f16)

            # ---- DMA issue.  All inputs go on the sync HWDGE ring in strict
            #      priority order (the ring drains FIFO, so this controls the
            #      wire): fcw(c0) -> x(c0) -> U(o0,c0) -> x(c1) -> ... with
            #      o0's U split per-c so the conv can start as x trickles in.
            #      st/fb (tiny) and the y stores ride the scalar ring. ----
            xn = singles.tile([128, PC, NPC, H, W], f16)
            nc.sync.dma_start(pk_sb[:], pk)
            nc.sync.dma_start(xn[:, 0], xr[0])
            nc.sync.dma_start(u_sb[:, 0, 0:2], ur[0][:, 0:2])
            nc.sync.dma_start(xn[:, 1], xr[1])
            nc.sync.dma_start(xn[:, 2], xr[2])
            nc.sync.dma_start(xn[:, 3], xr[3])
            nc.sync.dma_start(u_sb[:, 0, 2:3], ur[0][:, 2:3])
            nc.sync.dma_start(A_sb[:], ar)
            nc.sync.dma_start(u_sb[:, 0, 3:4], ur[0][:, 3:4])
            for o in range(1, OCC):
                nc.sync.dma_start(u_sb[:, o, 0:2], ur[o][:, 0:2])
                nc.sync.dma_start(u_sb[:, o, 2:4], ur[o][:, 2:4])

            nc.vector.memset(eps_sb[:], float(DEMOD_EPS))
            nc.vector.memset(warm_sb[:], 0.0)
            # zero only the pad border strips; the ACT x-mods write the
            # interior, so they carry no dependency on these
            for c in range(PC):
                for n in range(NPC):
                    nc.gpsimd.memset(xpad[:, c, n, 0], 0.0)
                    nc.gpsimd.memset(xpad[:, c, n, 33], 0.0)
                    nc.gpsimd.memset(xpad[:, c, n, :, 0], 0.0)
                    nc.gpsimd.memset(xpad[:, c, n, :, 33], 0.0)
            # dummy sqrt: pulls the ACT table load into the DMA-wait window
            nc.scalar.activation(sqd_sb[:], eps_sb[:],
                                 mybir.ActivationFunctionType.Sqrt)

            # ---- PE warm-up: ~3.4us of dummy matmuls so the HAM clock gate
            #      reaches 8/8 just as the real stream arrives ----
            wps = pss.tile([128, 512], f32, tag="pmm")
            NWARM = 8
            for i in range(NWARM):
                nc.tensor.matmul(wps[:], warm_sb[:, 0:128], warm_sb[:],
                                 start=(i == 0), stop=(i == NWARM - 1))

            # ---- style modulation: modT[ic, n] ----
            for c in range(PC):
                pm = pss.tile([128, NPC], f32, tag="pmm")
                for sc in range(SC):
                    nc.tensor.matmul(
                        pm[:], pk_sb[:, sc * 512 + c * 128:sc * 512 + (c + 1) * 128],
                        pk_sb[:, 2048 + sc * 2:2048 + sc * 2 + 2],
                        start=(sc == 0), stop=(sc == SC - 1))
                nc.scalar.activation(
                    modT_sb[:, c], pm[:], mybir.ActivationFunctionType.Identity,
                    bias=fbv[:, c:c + 1], scale=FC_SCALER)
            for c in range(PC):
                nc.vector.tensor_mul(mod2T_sb[:, c], modT_sb[:, c],
                                     modT_sb[:, c])

            # ---- bridge warm-up: keep the PE HAM-busy across the gap
            #      between the fc matmuls and the first conv group ----
            wps2 = pss.tile([128, 512], f32, tag="pmm")
            NWARMB = 14
            for i in range(NWARMB):
                nc.tensor.matmul(wps2[:], warm_sb[:, 0:128], warm_sb[:],
                                 start=(i == 0), stop=(i == NWARMB - 1))

            # ---- padded, modulated images + winograd input transform ----
            # original row g lands at padded row g+1 = 2*r2 + rp:
            #   g even -> (r2, rp) = (g/2, 1);  g odd -> ((g+1)/2, 0)
            for c in range(PC):
                for n in range(NPC):
                    m = modT_sb[:, c, n:n + 1]
                    nc.scalar.mul(xpad[:, c, n, 1:33, 1:33], xn[:, c, n], m)
                # d_i[n, ty, col] = xpad row 2*ty + i  (rows refactored
                # (r2, rp): row = 2*r2 + rp)
                xv = xpad[:, c].rearrange("p n (r2 rp) w -> p n r2 rp w", rp=2)
                d0 = xv[:, :, 0:16, 0]
                d1 = xv[:, :, 0:16, 1]
                d2 = xv[:, :, 1:17, 0]
                d3 = xv[:, :, 1:17, 1]
                nc.vector.tensor_sub(v_sb[:, c, 0], d0, d2)
                nc.vector.tensor_add(v_sb[:, c, 1], d1, d2)
                nc.vector.tensor_sub(v_sb[:, c, 2], d2, d1)
                nc.vector.tensor_sub(v_sb[:, c, 3], d1, d3)

            def demod_block():
                # sumsq matmuls + 1/sqrt(sumsq + eps'): emitted between the
                # first conv group's matmuls and its drains, so the sqrt ops
                # precede every drain in the ACT FIFO (the drains read
                # demodT) while the late A arrival can't stall the PE FIFO
                dps = pss.tile([128, OCC, NPC], f32, tag="pmm")
                for o in range(OCC):
                    for c in range(PC):
                        nc.tensor.matmul(
                            dps[:, o], A_sb[:, c, o * 128:(o + 1) * 128],
                            mod2T_sb[:, c], start=(c == 0), stop=(c == PC - 1))
                for o in range(OCC):
                    nc.scalar.activation(sq_sb[:, o], dps[:, o],
                                         mybir.ActivationFunctionType.Sqrt,
                                         bias=eps_sb[:])
                nc.vector.reciprocal(demodT_sb[:], sq_sb[:])
                nc.vector.tensor_scalar_mul(demodNT_sb[:], demodT_sb[:], -1.0)

            # ---- the conv: per (o, u) accumulate M(u) for both samples over
            #      (c, kx); n innermost so each LDWEIGHTS feeds 2 matmuls.
            #      Output transform consumes each M plane straight from PSUM:
            #        u=0: y0  = M0*dm (ACT)      u=1: y1  = M1*dm (ACT)
            #                                         y0 += M1*dm (DVE stt)
            #        u=2: y0 += M2*dm, y1 -= M2*dm    u=3: y1 -= M3*dm ----
            mult, add = mybir.AluOpType.mult, mybir.AluOpType.add

            def consume(o, u, psn, n, ystage):
                # psn: [128, 512] PSUM plane M(u) for sample n
                dm = demodT_sb[:, o, n:n + 1]
                dmn = demodNT_sb[:, o, n:n + 1]
                y0 = ystage[:, n, :, 0]
                y1 = ystage[:, n, :, 1]
                if u == 0:
                    nc.scalar.mul(y0, psn, dm)
                elif u == 1:
                    nc.scalar.mul(y1, psn, dm)
                    nc.vector.scalar_tensor_tensor(
                        y0, psn, dm, y0, op0=mult, op1=add)
                elif u == 2:
                    nc.vector.scalar_tensor_tensor(
                        y0, psn, dm, y0, op0=mult, op1=add)
                    nc.vector.scalar_tensor_tensor(
                        y1, psn, dmn, y1, op0=mult, op1=add)
                else:
                    nc.vector.scalar_tensor_tensor(
                        y1, psn, dmn, y1, op0=mult, op1=add)

            # o = 0: c-major over u-pairs so the PE fills with (u, kx) work
            # while the later x chunks are still on the wire
            ystage0 = ypool.tile([128, NPC, TY, 2, W], f16, tag="yb")
            for pair in ((0, 1), (2, 3)):
                pps = {}
                for u in pair:
                    pps[u] = psc.tile([128, NPC, 512], f32, tag="ps",
                                      name=f"pp{u}")
                for c in range(PC):
                    for kx in range(K):
                        first = (c == 0 and kx == 0)
                        last = (c == PC - 1 and kx == K - 1)
                        for u in pair:
                            for n in range(NPC):
                                nc.tensor.matmul(
                                    pps[u][:, n],
                                    u_sb[:, 0, u, c, kx],
                                    v_sb[:, c, u, n, :, kx:kx + W],
                                    start=first, stop=last)
                    if pair == (0, 1) and c == PC - 2:
                        # the PE idles here waiting for the last x chunk;
                        # the demod matmuls ride in that stall for free, and
                        # a short dummy block keeps the HAM clock gate at
                        # 8/8 across the remainder of the wait
                        demod_block()
                        wps3 = pss.tile([128, 512], f32, tag="pmm")
                        for i in range(6):
                            nc.tensor.matmul(wps3[:], warm_sb[:, 0:128],
                                             warm_sb[:], start=(i == 0),
                                             stop=(i == 5))
                for u in pair:
                    for n in range(NPC):
                        consume(0, u, pps[u][:, n], n, ystage0)
            nc.scalar.dma_start(y[0], ystage0[:])

            # o = 1..3: u-serial groups (V fully resident; deep pipelining).
            # The very last group (o=3, u=3) is split by sample so the first
            # sample's output transform overlaps the second's matmuls.
            for o in range(1, OCC):
                ystage = ypool.tile([128, NPC, TY, 2, W], f16, tag="yb")
                for u in range(NU):
                    if o == OCC - 1 and u == NU - 1:
                        for n in range(NPC):
                            psn = psc.tile([128, 512], f32, tag="ps",
                                           name=f"psn{n}")
                            first = True
                            for c in range(PC):
                                for kx in range(K):
                                    last = (c == PC - 1 and kx == K - 1)
                                    nc.tensor.matmul(
                                        psn[:], u_sb[:, o, u, c, kx],
                                        v_sb[:, c, u, n, :, kx:kx + W],
                                        start=first, stop=last)
                                    first = False
                            consume(o, u, psn[:], n, ystage)
                            nc.scalar.dma_start(y[o][:, n, :, 1],
                                                ystage[:, n, :, 1])
                    else:
                        ps = psc.tile([128, NPC, 512], f32, tag="ps")
                        first = True
                        for c in range(PC):
                            for kx in range(K):
                                last = (c == PC - 1 and kx == K - 1)
                                for n in range(NPC):
                                    nc.tensor.matmul(
                                        ps[:, n],
                                        u_sb[:, o, u, c, kx],
                                        v_sb[:, c, u, n, :, kx:kx + W],
                                        start=first, stop=last)
                                first = False
                        for n in range(NPC):
                            consume(o, u, ps[:, n], n, ystage)
                        if o == OCC - 1 and u == NU - 2:
                            # y0 rows are final once u=2 is consumed
                            for n in range(NPC):
                                nc.scalar.dma_start(y[o][:, n, :, 0],
                                                    ystage[:, n, :, 0])
                if o != OCC - 1:
                    nc.scalar.dma_start(y[o], ystage[:])

    nc.finalize()
    _NC_CACHE["nc"] = nc
    return nc


def _shard_inputs(x, style, weight, fc_weight, fc_bias):
    f = np.float32
    w = weight.astype(f)                                     # [OC, IC, 3, 3]
    # F(2,3) weight transform along ky (host-side weight prepacking)
    G1 = np.array([[1, 0, 0], [.5, .5, .5], [.5, -.5, .5], [0, 0, 1]], f)
    U = np.einsum('uy,oiyx->iuxo', G1, w)                    # [IC, 4, 3, OC]
    # device layout [OCC, NU, IC, K, 128(oc)]
    ut_host = np.ascontiguousarray(
        U.reshape(IC, NU, K, OCC, 128).transpose(3, 1, 0, 2, 4).astype(np.float16))
    at_host = np.ascontiguousarray((w * w).sum(axis=(2, 3)).T.astype(np.float16))
    # pk: per-partition p: [fcw(sc, i) | st(sc, n) | (fcb+1) f32 as fp16 halves]
    fcw16 = fc_weight.astype(np.float16).T.reshape(SC, 128, IC)   # [sc, p, i]
    fb32 = (fc_bias.astype(f) + 1.0).reshape(PC, 128)             # [c, p]
    fbh = fb32.astype(f).view(np.float16).reshape(PC, 128, 2)     # [c, p, 2]
    in_maps = []
    for i in range(N_CORES):
        sl = slice(i * NPC, (i + 1) * NPC)
        st16 = style[sl].astype(np.float16).T.reshape(SC, 128, NPC)
        pk_host = np.empty((128, 2064), np.float16)
        pk_host[:, 0:2048] = fcw16.transpose(1, 0, 2).reshape(128, SC * IC)
        pk_host[:, 2048:2056] = st16.transpose(1, 0, 2).reshape(128, SC * NPC)
        pk_host[:, 2056:2064] = fbh.transpose(1, 0, 2).reshape(128, PC * 2)
        in_maps.append({
            "x": np.ascontiguousarray(x[sl].astype(np.float16)),
            "pk": np.ascontiguousarray(pk_host),
            "ut": ut_host,
            "at": at_host,
        })
    return in_maps


def kernel(x, style, weight, fc_weight, fc_bias):
    global LAST_RESULT
    x = np.asarray(x)
    style = np.asarray(style)
    weight = np.asarray(weight)
    fc_weight = np.asarray(fc_weight)
    fc_bias = np.asarray(fc_bias)

    nc = build_nc()
    in_maps = _shard_inputs(x, style, weight, fc_weight, fc_bias)
    res = bass_utils.run_bass_kernel_spmd(
        nc, in_maps, core_ids=list(range(N_CORES)))
    LAST_RESULT = res
    # y: [OCC, 128, NPC, TY, 2, W] fp16 -> [NPC, OC, H, W]
    out = np.concatenate(
        [res.results[i]["y"].transpose(2, 0, 1, 3, 4, 5).reshape(NPC, OC, H, W)
         for i in range(N_CORES)],
        axis=0)
    return out.astype(np.float32)


# revision 49
# speedup vs baseline: 1.0306x; 1.0306x over previous
"""EqualizedModulatedConv2d (StyleGAN2-style modulated conv) on 8 Trainium2 cores.

Reference computation (per sample n):
    mod[n, ic]  = (style[n] @ fc_weight.T) * FC_SCALER + fc_bias + 1
    w[n]        = WEIGHT_SCALER * weight * mod[n, :, None, None]          # [oC, iC, 3, 3]
    demod[n,oc] = rsqrt(sum_{ic,kh,kw} w^2 + 1e-8)
    out[n]      = conv2d(x[n], w[n] * demod[n, :, None, None, None], pad=1)

Identities used on device: the conv is linear, so
    out[n, oc] = (WEIGHT_SCALER * demod[n, oc]) * conv2d(x[n] * mod[n, ic], weight)
with
    WEIGHT_SCALER * demod[n, oc] = 1 / sqrt(sumsq[n, oc] + 1e-8 / WEIGHT_SCALER^2)
    sumsq[n, oc] = sum_ic A[ic, oc] * mod[n, ic]^2,   A[ic, oc] = sum_taps weight^2

The conv itself runs as 1-D Winograd F(2,3) along H: for output row-pair ty
(rows 2ty, 2ty+1) and padded-image rows d_i = xpad[2ty+i] (i = 0..3),
    V0 = d0 - d2, V1 = d1 + d2, V2 = d2 - d1, V3 = d1 - d3          (DVE)
    M(u)[oc, ty, x] = sum_{ic, kx} U(u,kx)[ic, oc] * V(u)[ic, ty, x+kx]   (PE)
    out[2ty+0] = M0 + M1 + M2,  out[2ty+1] = M1 - M2 - M3           (ACT+DVE)
where U(u,kx) = sum_ky G[u,ky] w[oc, ic, ky, kx] (G = F(2,3) weight
transform) is pre-packed on the host.  12 shifted matmuls per 2 output
rows instead of 18 -> 1.5x less PE work than direct conv (the PE streams
1 elem/cycle regardless of dtype, so MAC count is the only lever).
Everything flows in fp16; PSUM accumulates fp32.  The output transform
consumes PSUM incrementally (ACT mul for the first use of each M plane,
DVE scalar_tensor_tensor for the accumulating uses) so no M staging is
needed and the drain tail is short.

Sharding: data-parallel over N (16 samples / 8 cores = 2 per core);
weights replicated per core.  DMAs are batched (>=0.5 MB) and split
across the two HWDGE issue queues (sync + scalar) to dodge the ~0.64us
per-issue serialization observed in the trace.
"""

import numpy as np

import concourse.bass as bass
import concourse.tile as tile
from concourse import bacc, mybir
import concourse.bass_utils as bass_utils

# keep profiling artifacts local — no S3 in the sandbox
bass_utils.upload_artifacts = lambda tmpdir: "local://" + str(tmpdir)

# ---- problem constants (hardcoded per the harness contract) ----
N, IC, OC, K, SDIM, H, W = 16, 512, 512, 3, 512, 32, 32
N_CORES = 8
NPC = N // N_CORES            # samples per core = 2
PC = IC // 128                # ic chunks = 4
OCC = OC // 128               # oc chunks = 4
SC = SDIM // 128              # sdim chunks = 4
NTAP = 12                     # 4 winograd u-taps x 3 kx taps
NU = 4                        # winograd taps along H
TY = H // 2                   # output row-pair tiles = 16
HP, WP = H + 2, W + 2         # 34, 34 padded
FC_SCALER = 1.0 / np.sqrt(SDIM)
WEIGHT_SCALER = 1.0 / np.sqrt(IC * K * K)
DEMOD_EPS = 1e-8 / (WEIGHT_SCALER * WEIGHT_SCALER)   # 1e-8 * IC * K * K

MODE = "wino1d-fp16-v17"

_NC_CACHE = {}
LAST_RESULT = None  # test.py reads exec_time_ns off this


def build_nc():
    if "nc" in _NC_CACHE:
        return _NC_CACHE["nc"]

    f32 = mybir.dt.float32
    f16 = mybir.dt.float16

    nc = bacc.Bacc("TRN2", target_bir_lowering=False, debug=False,
                   num_devices=N_CORES)

    x = nc.dram_tensor("x", [NPC, IC, H, W], f16, kind="ExternalInput").ap()
    # pk: per-partition [fcw(sc,i) 2048 | st(sc,n) 8 | fcb-as-f32-halves 8]
    pk = nc.dram_tensor("pk", [128, 2064], f16, kind="ExternalInput").ap()
    ut = nc.dram_tensor("ut", [OCC, NU, IC, K, 128], f16, kind="ExternalInput").ap()
    at = nc.dram_tensor("at", [IC, OC], f16, kind="ExternalInput").ap()
    y = nc.dram_tensor("y", [OCC, 128, NPC, TY, 2, W], f16,
                       kind="ExternalOutput").ap()

    xr = x.rearrange("n (c p) g w -> c p n g w", p=128)
    ur = ut.rearrange("o u (c p) t q -> o p u c t q", p=128)
    ar = at.rearrange("(c p) o -> p c o", p=128)

    with tile.TileContext(nc) as tc:
        import contextlib
        with contextlib.ExitStack() as ctx:
            singles = ctx.enter_context(tc.tile_pool(name="singles", bufs=1))
            ypool = ctx.enter_context(tc.tile_pool(name="ypool", bufs=2))
            psc = ctx.enter_context(tc.tile_pool(name="psc", bufs=3, space="PSUM"))
            pss = ctx.enter_context(tc.tile_pool(name="pss", bufs=2, space="PSUM"))

            # ---- persistent SBUF tensors ----
            pk_sb = singles.tile([128, 2064], f16)
            modT_sb = singles.tile([128, PC, NPC], f32)
            mod2T_sb = singles.tile([128, PC, NPC], f16)
            A_sb = singles.tile([128, PC, OC], f16)
            demodT_sb = singles.tile([128, OCC, NPC], f32)
            demodNT_sb = singles.tile([128, OCC, NPC], f32)
            u_sb = singles.tile([128, OCC, NU, PC, K, 128], f16)
            fbv = pk_sb[:, 2056:2064].bitcast(f32)
            # padded modulated image, flat rows (views refactor as needed)
            xpad = singles.tile([128, PC, NPC, HP, WP], f16)
            # winograd-transformed input: [c][u][n][ty][col]
            v_sb = singles.tile([128, PC, NU, NPC, TY, WP], f16)
            eps_sb = singles.tile([128, 1], f32)
            sqd_sb = singles.tile([128, 1], f32)
            sq_sb = singles.tile([128, OCC, NPC], f32)
            warm_sb = singles.tile([128, 512], f16)

            # ---- DMA issue.  All inputs go on the sync HWDGE ring in strict
            #      priority order (the ring drains FIFO, so this controls the
            #      wire): fcw(c0) -> x(c0) -> U(o0,c0) -> x(c1) -> ... with
            #      o0's U split per-c so the conv can start as x trickles in.
            #      st/fb (tiny) and the y stores ride the scalar ring. ----
            xn = singles.tile([128, PC, NPC, H, W], f16)
            nc.sync.dma_start(pk_sb[:], pk)
            nc.sync.dma_start(xn[:, 0], xr[0])
            nc.sync.dma_start(u_sb[:, 0, 0:2], ur[0][:, 0:2])
            nc.sync.dma_start(xn[:, 1], xr[1])
            nc.sync.dma_start(xn[:, 2], xr[2])
            nc.sync.dma_start(xn[:, 3], xr[3])
            nc.sync.dma_start(u_sb[:, 0, 2:3], ur[0][:, 2:3])
            nc.sync.dma_start(A_sb[:], ar)
            nc.sync.dma_start(u_sb[:, 0, 3:4], ur[0][:, 3:4])
            for o in range(1, OCC):
                nc.sync.dma_start(u_sb[:, o, 0:2], ur[o][:, 0:2])
                nc.sync.dma_start(u_sb[:, o, 2:4], ur[o][:, 2:4])

            nc.vector.memset(eps_sb[:], float(DEMOD_EPS))
            nc.vector.memset(warm_sb[:], 0.0)
            # zero only the pad border strips; the ACT x-mods write the
            # interior, so they carry no dependency on these
            for c in range(PC):
                for n in range(NPC):
                    nc.gpsimd.memset(xpad[:, c, n, 0], 0.0)
                    nc.gpsimd.memset(xpad[:, c, n, 33], 0.0)
                    nc.gpsimd.memset(xpad[:, c, n, :, 0], 0.0)
                    nc.gpsimd.memset(xpad[:, c, n, :, 33], 0.0)
            # dummy sqrt: pulls the ACT table load into the DMA-wait window
            nc.scalar.activation(sqd_sb[:], eps_sb[:],
                                 mybir.ActivationFunctionType.Sqrt)

            # ---- PE warm-up: ~3.4us of dummy matmuls so the HAM clock gate
            #      reaches 8/8 just as the real stream arrives ----
            wps = pss.tile([128, 512], f32, tag="pmm")
            NWARM = 8
            for i in range(NWARM):
                nc.tensor.matmul(wps[:], warm_sb[:, 0:128], warm_sb[:],
                                 start=(i == 0), stop=(i == NWARM - 1))

            # ---- style modulation: modT[ic, n] ----
            for c in range(PC):
                pm = pss.tile([128, NPC], f32, tag="pmm")
                for sc in range(SC):
                    nc.tensor.matmul(
                        pm[:], pk_sb[:, sc * 512 + c * 128:sc * 512 + (c + 1) * 128],
                        pk_sb[:, 2048 + sc * 2:2048 + sc * 2 + 2],
                        start=(sc == 0), stop=(sc == SC - 1))
                nc.scalar.activation(
                    modT_sb[:, c], pm[:], mybir.ActivationFunctionType.Identity,
                    bias=fbv[:, c:c + 1], scale=FC_SCALER)
            for c in range(PC):
                nc.vector.tensor_mul(mod2T_sb[:, c], modT_sb[:, c],
                                     modT_sb[:, c])

            # ---- bridge warm-up: keep the PE HAM-busy across the gap
            #      between the fc matmuls and the first conv group ----
            wps2 = pss.tile([128, 512], f32, tag="pmm")
            NWARMB = 10
            for i in range(NWARMB):
                nc.tensor.matmul(wps2[:], warm_sb[:, 0:128], warm_sb[:],
                                 start=(i == 0), stop=(i == NWARMB - 1))

            # ---- padded, modulated images + winograd input transform ----
            # original row g lands at padded row g+1 = 2*r2 + rp:
            #   g even -> (r2, rp) = (g/2, 1);  g odd -> ((g+1)/2, 0)
            for c in range(PC):
                for n in range(NPC):
                    m = modT_sb[:, c, n:n + 1]
                    nc.scalar.mul(xpad[:, c, n, 1:33, 1:33], xn[:, c, n], m)
                # d_i[n, ty, col] = xpad row 2*ty + i  (rows refactored
                # (r2, rp): row = 2*r2 + rp)
                xv = xpad[:, c].rearrange("p n (r2 rp) w -> p n r2 rp w", rp=2)
                d0 = xv[:, :, 0:16, 0]
                d1 = xv[:, :, 0:16, 1]
                d2 = xv[:, :, 1:17, 0]
                d3 = xv[:, :, 1:17, 1]
                nc.vector.tensor_sub(v_sb[:, c, 0], d0, d2)
                nc.vector.tensor_add(v_sb[:, c, 1], d1, d2)
                nc.vector.tensor_sub(v_sb[:, c, 2], d2, d1)
                nc.vector.tensor_sub(v_sb[:, c, 3], d1, d3)

            def demod_block():
                # sumsq matmuls + 1/sqrt(sumsq + eps'): emitted between the
                # first conv group's matmuls and its drains, so the sqrt ops
                # precede every drain in the ACT FIFO (the drains read
                # demodT) while the late A arrival can't stall the PE FIFO
                dps = pss.tile([128, OCC, NPC], f32, tag="pmm")
                for o in range(OCC):
                    for c in range(PC):
                        nc.tensor.matmul(
                            dps[:, o], A_sb[:, c, o * 128:(o + 1) * 128],
                            mod2T_sb[:, c], start=(c == 0), stop=(c == PC - 1))
                for o in range(OCC):
                    nc.scalar.activation(sq_sb[:, o], dps[:, o],
                                         mybir.ActivationFunctionType.Sqrt,
                                         bias=eps_sb[:])
                nc.vector.reciprocal(demodT_sb[:], sq_sb[:])
                nc.vector.tensor_scalar_mul(demodNT_sb[:], demodT_sb[:], -1.0)

            # ---- the conv: per (o, u) accumulate M(u) for both samples over
            #      (c, kx); n innermost so each LDWEIGHTS feeds 2 matmuls.
            #      Output transform consumes each M plane straight from PSUM:
            #        u=0: y0  = M0*dm (ACT)      u=1: y1  = M1*dm (ACT)
            #                                         y0 += M1*dm (DVE stt)
            #        u=2: y0 += M2*dm, y1 -= M2*dm    u=3: y1 -= M3*dm ----
            mult, add = mybir.AluOpType.mult, mybir.AluOpType.add

            def consume(o, u, psn, n, ystage):
                # psn: [128, 512] PSUM plane M(u) for sample n
                dm = demodT_sb[:, o, n:n + 1]
                dmn = demodNT_sb[:, o, n:n + 1]
                y0 = ystage[:, n, :, 0]
                y1 = ystage[:, n, :, 1]
                if u == 0:
                    nc.scalar.mul(y0, psn, dm)
                elif u == 1:
                    nc.scalar.mul(y1, psn, dm)
                    nc.vector.scalar_tensor_tensor(
                        y0, psn, dm, y0, op0=mult, op1=add)
                elif u == 2:
                    nc.vector.scalar_tensor_tensor(
                        y0, psn, dm, y0, op0=mult, op1=add)
                    nc.vector.scalar_tensor_tensor(
                        y1, psn, dmn, y1, op0=mult, op1=add)
                else:
                    nc.vector.scalar_tensor_tensor(
                        y1, psn, dmn, y1, op0=mult, op1=add)

            # o = 0: c-major over u-pairs so the PE fills with (u, kx) work
            # while the later x chunks are still on the wire
            ystage0 = ypool.tile([128, NPC, TY, 2, W], f16, tag="yb")
            for pair in ((0, 1), (2, 3)):
                pps = {}
                for u in pair:
                    pps[u] = psc.tile([128, NPC, 512], f32, tag="ps",
                                      name=f"pp{u}")
                for c in range(PC):
                    for kx in range(K):
                        first = (c == 0 and kx == 0)
                        last = (c == PC - 1 and kx == K - 1)
                        for u in pair:
                            for n in range(NPC):
                                nc.tensor.matmul(
                                    pps[u][:, n],
                                    u_sb[:, 0, u, c, kx],
                                    v_sb[:, c, u, n, :, kx:kx + W],
                                    start=first, stop=last)
                    if pair == (0, 1) and c == PC - 2:
                        # the PE idles here waiting for the last x chunk;
                        # the demod matmuls ride in that stall for free, and
                        # a short dummy block keeps the HAM clock gate at
                        # 8/8 across the remainder of the wait
                        demod_block()
                        wps3 = pss.tile([128, 512], f32, tag="pmm")
                        for i in range(6):
                            nc.tensor.matmul(wps3[:], warm_sb[:, 0:128],
                                             warm_sb[:], start=(i == 0),
                                             stop=(i == 5))
                for u in pair:
                    for n in range(NPC):
                        consume(0, u, pps[u][:, n], n, ystage0)
            nc.scalar.dma_start(y[0], ystage0[:])

            # o = 1..3: u-serial groups (V fully resident; deep pipelining).
            # The very last group (o=3, u=3) is split by sample so the first
            # sample's output transform overlaps the second's matmuls.
            for o in range(1, OCC):
                ystage = ypool.tile([128, NPC, TY, 2, W], f16, tag="yb")
                for u in range(NU):
                    if o == OCC - 1 and u == NU - 1:
                        for n in range(NPC):
                            psn = psc.tile([128, 512], f32, tag="ps",
                                           name=f"psn{n}")
                            first = True
                            for c in range(PC):
                                for kx in range(K):
                                    last = (c == PC - 1 and kx == K - 1)
                                    nc.tensor.matmul(
                                        psn[:], u_sb[:, o, u, c, kx],
                                        v_sb[:, c, u, n, :, kx:kx + W],
                                        start=first, stop=last)
                                    first = False
                            consume(o, u, psn[:], n, ystage)
                            nc.scalar.dma_start(y[o][:, n, :, 1],
                                                ystage[:, n, :, 1])
                    else:
                        ps = psc.tile([128, NPC, 512], f32, tag="ps")
                        first = True
                        for c in range(PC):
                            for kx in range(K):
                                last = (c == PC - 1 and kx == K - 1)
                                for n in range(NPC):
                                    nc.tensor.matmul(
                                        ps[:, n],
                                        u_sb[:, o, u, c, kx],
                                        v_sb[:, c, u, n, :, kx:kx + W],
                                        start=first, stop=last)
                                first = False
                        for n in range(NPC):
                            consume(o, u, ps[:, n], n, ystage)
                        if o == OCC - 1 and u == NU - 2:
                            # y0 rows are final once u=2 is consumed
                            for n in range(NPC):
                                nc.scalar.dma_start(y[o][:, n, :, 0],
                                                    ystage[:, n, :, 0])
                if o != OCC - 1:
                    nc.scalar.dma_start(y[o], ystage[:])

    nc.finalize()
    _NC_CACHE["nc"] = nc
    return nc


def _shard_inputs(x, style, weight, fc_weight, fc_bias):
    f = np.float32
    w = weight.astype(f)                                     # [OC, IC, 3, 3]
    # F(2,3) weight transform along ky (host-side weight prepacking)
    G1 = np.array([[1, 0, 0], [.5, .5, .5], [.5, -.5, .5], [0, 0, 1]], f)
    U = np.einsum('uy,oiyx->iuxo', G1, w)                    # [IC, 4, 3, OC]
    # device layout [OCC, NU, IC, K, 128(oc)]
    ut_host = np.ascontiguousarray(
        U.reshape(IC, NU, K, OCC, 128).transpose(3, 1, 0, 2, 4).astype(np.float16))
    at_host = np.ascontiguousarray((w * w).sum(axis=(2, 3)).T.astype(np.float16))
    # pk: per-partition p: [fcw(sc, i) | st(sc, n) | (fcb+1) f32 as fp16 halves]
    fcw16 = fc_weight.astype(np.float16).T.reshape(SC, 128, IC)   # [sc, p, i]
    fb32 = (fc_bias.astype(f) + 1.0).reshape(PC, 128)             # [c, p]
    fbh = fb32.astype(f).view(np.float16).reshape(PC, 128, 2)     # [c, p, 2]
    in_maps = []
    for i in range(N_CORES):
        sl = slice(i * NPC, (i + 1) * NPC)
        st16 = style[sl].astype(np.float16).T.reshape(SC, 128, NPC)
        pk_host = np.empty((128, 2064), np.float16)
        pk_host[:, 0:2048] = fcw16.transpose(1, 0, 2).reshape(128, SC * IC)
        pk_host[:, 2048:2056] = st16.transpose(1, 0, 2).reshape(128, SC * NPC)
        pk_host[:, 2056:2064] = fbh.transpose(1, 0, 2).reshape(128, PC * 2)
        in_maps.append({
            "x": np.ascontiguousarray(x[sl].astype(np.float16)),
            "pk": np.ascontiguousarray(pk_host),
            "ut": ut_host,
            "at": at_host,
        })
    return in_maps


def kernel(x, style, weight, fc_weight, fc_bias):
    global LAST_RESULT
    x = np.asarray(x)
    style = np.asarray(style)
    weight = np.asarray(weight)
    fc_weight = np.asarray(fc_weight)
    fc_bias = np.asarray(fc_bias)

    nc = build_nc()
    in_maps = _shard_inputs(x, style, weight, fc_weight, fc_bias)
    res = bass_utils.run_bass_kernel_spmd(
        nc, in_maps, core_ids=list(range(N_CORES)))
    LAST_RESULT = res
    # y: [OCC, 128, NPC, TY, 2, W] fp16 -> [NPC, OC, H, W]
    out = np.concatenate(
        [res.results[i]["y"].transpose(2, 0, 1, 3, 4, 5).reshape(NPC, OC, H, W)
         for i in range(N_CORES)],
        axis=0)
    return out.astype(np.float32)
